# revision 5
# baseline (speedup 1.0000x reference)
"""Trainium2 Bass kernel v2 for the dense transformer layer (RMSNorm -> GQA
attention -> RMSNorm -> SwiGLU MLP, residuals, RoPE).  b=16,s=512,hid=2048,
nq=32,nkv=8,hd=64,inter=8192, fp32 I/O.

Sharding: data-parallel over batch -- 2 batch elements (1024 tokens) per core
across 8 NeuronCores, no collectives.

v2 changes vs v1:
- Weights shipped as bf16, pre-tiled host-side into DMA-contiguous slabs in
  exact consumption order (no on-chip f32->bf16 DVE conversion, half the DMA).
- ln1/ln2 folded into weight rows host-side.
- o-proj and down-proj produce token-major outputs directly (activation tile
  as lhsT, weight as rhs), accumulating in PSUM: no DRAM accumulator, no
  final transpose phase, residuals added at PSUM drain.
- Attention: one 2048-wide exp per (head, batch); unnormalized ctx + denom
  drained together; softmax renorm deferred to a per-head pass (PE broadcast
  of reciprocal denominators).
- RoPE rotate-halves done as fused swap-multiplies split across DVE and Pool.
- qT/ctxT/M stay SBUF-resident (no DRAM round-trips); res1 (token-major,
  f32) is the only DRAM scratch.
"""

import sys
import numpy as np

sys.path.insert(0, "/opt/trn_rl_repo")

import concourse.bass as bass  # noqa: E402
import concourse.tile as tile  # noqa: E402
from concourse import mybir  # noqa: E402

F32 = mybir.dt.float32
F32R = mybir.dt.float32r
BF16 = mybir.dt.bfloat16
MULT = mybir.AluOpType.mult
ADD = mybir.AluOpType.add
AF = mybir.ActivationFunctionType

N_CORES = 8
B, S, HID = 16, 512, 2048
NQ, NKV, HD, INTER = 32, 8, 64, 8192
T = (B // N_CORES) * S  # tokens per core = 1024
BPC = B // N_CORES      # batch elements per core = 2
KT = HID // 128         # 16 k-tiles of hidden
IC = INTER // 128       # 64 inter col-tiles
TC8 = T // 128          # 8 token chunks
EPS = 1e-6
ROPE_BASE = 10000.0

MAXW = 1  # max sync waits per instruction


def _split_waits(nc):
    k = 0
    for f in nc.m.functions:
        for blk in f.blocks:
            newlist, changed = [], False
            for i in blk.instructions:
                si = i.sync_info
                if si is not None and len(si.on_wait) > MAXW:
                    waits = list(si.on_wait)
                    for w in waits[:-MAXW]:
                        k += 1
                        nop = mybir.InstNoOp(name=f"ws_{k}", ins=[], outs=[])
                        nop.engine = i.engine
                        nop.sync_info = mybir.SyncInfo(on_wait=[w], on_update=[])
                        newlist.append(nop)
                    i.sync_info = mybir.SyncInfo(
                        on_wait=waits[-MAXW:], on_update=list(si.on_update))
                    changed = True
                newlist.append(i)
            if changed:
                blk.instructions = newlist


def build(reps: int = 1, sim_compat: bool = False, upto: int = 9):
    nc = bass.Bass("TRN2", target_bir_lowering=False, debug=False,
                   num_devices=N_CORES)

    x_d = nc.dram_tensor("x", (T, HID), F32, kind="ExternalInput")
    # pre-tiled bf16 weights, [block_row, 512/2048 cols] slabs:
    wqkv_d = nc.dram_tensor("wqkvt", (6 * KT * 128, 512), BF16,
                            kind="ExternalInput")
    wo_d = nc.dram_tensor("wot", (4 * KT * 128, 512), BF16,
                          kind="ExternalInput")
    wg_d = nc.dram_tensor("wgt", (IC * 128, 2048), BF16, kind="ExternalInput")
    wu_d = nc.dram_tensor("wut", (IC * 128, 2048), BF16, kind="ExternalInput")
    wd_d = nc.dram_tensor("wdt", (4 * IC * 128, 512), BF16,
                          kind="ExternalInput")
    cos_d = nc.dram_tensor("cos128", (128, T), F32, kind="ExternalInput")
    sin_d = nc.dram_tensor("sinS128", (128, T), F32, kind="ExternalInput")
    identb_d = nc.dram_tensor("identb", (128, 128), BF16, kind="ExternalInput")
    onesm_d = nc.dram_tensor("onesm64", (1, 64), F32R, kind="ExternalInput")
    eps_d = nc.dram_tensor("eps", (128, 1), F32, kind="ExternalInput")
    out_d = nc.dram_tensor("out", (T, HID), F32, kind="ExternalOutput")

    with tile.TileContext(nc) as tc:
        consts_p = tc.tile_pool(name="consts", bufs=1)
        consts = consts_p.__enter__()
        dram_p = tc.tile_pool(name="drscr", bufs=1, space="DRAM")
        drs = dram_p.__enter__()

        identb = consts.tile([128, 128], BF16)
        nc.sync.dma_start(identb, identb_d[:, :])
        onesm = consts.tile([1, 64], F32R)
        nc.sync.dma_start(onesm, onesm_d[:, :])
        epst = consts.tile([128, 1], F32)
        nc.sync.dma_start(epst, eps_d[:, :])
        cos128 = consts.tile([128, T], F32)
        nc.sync.dma_start(cos128, cos_d[:, :])
        sinS = consts.tile([128, T], F32)
        nc.sync.dma_start(sinS, sin_d[:, :])

        res1_dram = drs.tile([T, HID], F32, name="res1_scr")

        def norm_transpose(src_getter, dst_all, pool, psp, src_f32_bytes):
            """token-major src chunks -> rmsnorm -> feat-major dst tiles.
            src_getter(i) -> (dma_src_ap) for chunk i; dst_all [128, KT*T]."""
            for i in range(TC8):
                x_t = pool.tile([128, HID], F32, name="nx")
                nc.sync.dma_start(x_t, src_getter(i))
                scr = pool.tile([128, HID], BF16, name="nscr")
                ssq = pool.tile([128, 1], F32, name="nssq")
                nc.scalar.activation(scr, x_t, AF.Square, accum_out=ssq)
                istd = pool.tile([128, 1], F32, name="nistd")
                nc.scalar.activation(istd, ssq, AF.Sqrt, bias=epst,
                                     scale=1.0 / HID)
                with nc.allow_low_precision("rms inv-std"):
                    nc.vector.reciprocal(istd, istd)
                h_b = pool.tile([128, HID], BF16, name="nh")
                nc.vector.tensor_scalar(h_b, x_t, istd, None, MULT)
                for jg in range(4):
                    tp = psp.tile([128, 512], BF16, name="ntp")
                    for j4 in range(4):
                        j = jg * 4 + j4
                        nc.tensor.transpose(tp[:, j4 * 128:(j4 + 1) * 128],
                                            h_b[:, j * 128:(j + 1) * 128],
                                            identb)
                    dst = dst_all.rearrange("p (kt t) -> p kt t", kt=KT)
                    nc.scalar.copy(
                        dst[:, jg * 4:jg * 4 + 4, i * 128:(i + 1) * 128],
                        tp.rearrange("p (a b) -> p a b", a=4))

        def body():
            # pool stack (LIFO release): ctxp (thru P6) > kvp (thru P5) >
            # vfp (thru P4) > hTp (thru P3)
            ctx_p = tc.tile_pool(name="ctxp", bufs=1)
            ctxl = ctx_p.__enter__()
            ctxT = ctxl.tile([128, KT * T], BF16, name="ctxT")
            kv_p = tc.tile_pool(name="kvp", bufs=1)
            kvl = kv_p.__enter__()
            qT = kvl.tile([128, KT * T], BF16, name="qT")
            kTdup = [kvl.tile([128, T], BF16, name=f"kTd{j}")
                     for j in range(NKV)]
            v65 = kvl.tile([128, TC8, NKV, 65], BF16, name="v65")
            vf_p = tc.tile_pool(name="vfp", bufs=1)
            vfl = vf_p.__enter__()
            vf = [vfl.tile([128, T], BF16, name=f"vf{j}") for j in range(4)]
            # ---- P1: x -> hT (feat-major bf16), ln1 folded into wqkv ----
            hT_p = tc.tile_pool(name="hTp", bufs=1)
            hTl = hT_p.__enter__()
            hT = hTl.tile([128, KT * T], BF16, name="hT")
            if upto <= 8:
                # serialize reps in truncated builds: chain this rep's hT
                # writes behind the previous rep's out_d anchor writes
                with tc.tile_pool(name="serp", bufs=1) as serp:
                    g_ = serp.tile([128, 128], F32, name="serg")
                    nc.sync.dma_start(g_, out_d[0:128, 0:128])
                    nc.vector.tensor_copy(hT[:, 0:128], g_)
            with tc.tile_pool(name="p1t", bufs=3) as p1t, \
                 tc.tile_pool(name="p1ps", bufs=4, space="PSUM") as p1ps:
                norm_transpose(lambda i: x_d[i * 128:(i + 1) * 128, :],
                               hT, p1t, p1ps, 4)

            # ---- P3: QKV + RoPE; q/k/v feat-major ----
            with tc.tile_pool(name="p3t", bufs=2) as p3t, \
                 tc.tile_pool(name="p3w", bufs=4) as p3w, \
                 tc.tile_pool(name="p3ps", bufs=1, space="PSUM") as p3ps:
                for mg in range(6):
                    ps = [[p3ps.tile([128, 512], F32, name=f"qkv{mi}_{th}")
                           for th in range(2)] for mi in range(4)]
                    for k in range(KT):
                        wblk = p3w.tile([128, 512], BF16, name="wblk")
                        r0 = (mg * KT + k) * 128
                        nc.sync.dma_start(wblk, wqkv_d[r0:r0 + 128, :])
                        for mi in range(4):
                            for th in range(2):
                                nc.tensor.matmul(
                                    ps[mi][th],
                                    wblk[:, mi * 128:(mi + 1) * 128],
                                    hT[:, k * T + th * 512:
                                       k * T + (th + 1) * 512],
                                    start=(k == 0), stop=(k == KT - 1))
                    for mi in range(4):
                        m = mg * 4 + mi
                        for th in range(2):
                            tsl = slice(th * 512, (th + 1) * 512)
                            if m < 20:  # q/k: RoPE
                                qa = p3t.tile([128, 512], F32, name="qa")
                                nc.scalar.copy(qa, ps[mi][th])
                                t1 = p3t.tile([128, 512], F32, name="t1")
                                nc.vector.tensor_tensor(t1, qa, cos128[:, tsl],
                                                        MULT)
                                qsw = p3t.tile([128, 512], F32, name="qsw")
                                for b2 in range(4):
                                    src = slice((b2 ^ 1) * 32,
                                                (b2 ^ 1) * 32 + 32)
                                    dst = slice(b2 * 32, b2 * 32 + 32)
                                    eng = nc.gpsimd if b2 < 2 else nc.scalar
                                    if b2 < 2:
                                        eng.tensor_copy(qsw[dst], qa[src])
                                    else:
                                        nc.scalar.copy(qsw[dst], qa[src])
                                t2 = p3t.tile([128, 512], F32, name="t2")
                                nc.vector.tensor_tensor(t2, qsw,
                                                        sinS[:, tsl], MULT)
                                if m < 16:
                                    nc.vector.tensor_tensor(
                                        qT[:, m * T + th * 512:
                                           m * T + (th + 1) * 512],
                                        t1, t2, ADD)
                                else:
                                    for hh in range(2):
                                        kvh = 2 * (m - 16) + hh
                                        hs = slice(hh * 64, hh * 64 + 64)
                                        for half in range(2):
                                            nc.vector.tensor_tensor(
                                                kTdup[kvh][half * 64:
                                                           half * 64 + 64,
                                                           tsl],
                                                t1[hs], t2[hs], ADD)
                            else:
                                nc.scalar.copy(vf[m - 20][:, tsl], ps[mi][th])
            hT_p.__exit__(None, None, None)

            # ---- P4: v -> token-major v65 (with ones col for denom) ----
            with tc.tile_pool(name="p4ps", bufs=4, space="PSUM") as p4ps:
                # ones column via vf*0+1 (not memset) so it carries a dep on
                # this rep's vf and can't float ahead of the previous rep
                nc.vector.tensor_scalar(v65[:, :, :, 64:65], vf[0][:, 0:64],
                                        0.0, 1.0, MULT, ADD)
                for j in range(4):
                    for tci in range(TC8):
                        tp = p4ps.tile([128, 128], BF16, name="vtp")
                        nc.tensor.transpose(
                            tp, vf[j][:, tci * 128:(tci + 1) * 128], identb)
                        nc.scalar.copy(
                            v65[:, tci, 2 * j:2 * j + 2, 0:64],
                            tp.rearrange("p (a b) -> p a b", a=2))
            vf_p.__exit__(None, None, None)

            # ---- P5: attention; 256-query units, sw-pipelined by 1 ----
            with tc.tile_pool(name="p5t", bufs=3) as p5t, \
                 tc.tile_pool(name="p5u", bufs=3) as p5u, \
                 tc.tile_pool(name="p5psS", bufs=2, space="PSUM") as p5psS, \
                 tc.tile_pool(name="p5psC", bufs=2, space="PSUM") as p5psC, \
                 tc.tile_pool(name="p5psB", bufs=1, space="PSUM") as p5psB:
                def emit_renorm(cu, qh):
                    rden = p5t.tile([1, T], F32R, name="rden")
                    with nc.allow_low_precision("softmax denom"):
                        nc.vector.reciprocal(rden, cu[64:65, :])
                    bc_ps = p5psB.tile([64, T], F32, name="bcp")
                    for th in range(2):
                        nc.tensor.matmul(bc_ps[:, th * 512:(th + 1) * 512],
                                         onesm,
                                         rden[:, th * 512:(th + 1) * 512],
                                         start=True, stop=True)
                    nc.vector.tensor_tensor(
                        ctxT[(qh % 2) * 64:(qh % 2) * 64 + 64,
                             (qh // 2) * T:(qh // 2 + 1) * T],
                        cu[0:64, :], bc_ps, MULT)

                def emit_ctx(u):
                    (qh, b, h2), E, cu = u
                    kvh = qh // 4
                    ctx_ps = p5psC.tile([65, 256], F32, name="ctx")
                    for kc in range(4):
                        nc.tensor.matmul(ctx_ps,
                                         v65[:, b * 4 + kc, kvh, :],
                                         E[:, kc * 256:(kc + 1) * 256],
                                         start=(kc == 0), stop=(kc == 3))
                    q0 = b * 512 + h2 * 256
                    nc.vector.tensor_copy(cu[:, q0:q0 + 256], ctx_ps)

                units = [(qh, b, h2) for qh in range(NQ)
                         for b in range(BPC) for h2 in range(2)]
                prev = None
                pend_renorm = None
                cu = None
                for (qh, b, h2) in units:
                    if b == 0 and h2 == 0:
                        cu = p5u.tile([65, T], BF16, name="cu")
                    kvh = qh // 4
                    qrow = (qh % 2) * 64
                    qp = qh // 2
                    sc = p5psS.tile([128, 1024], F32, name="sc")
                    q0 = b * 512 + h2 * 256
                    for kc in range(4):
                        nc.tensor.matmul(
                            sc[:, kc * 256:(kc + 1) * 256],
                            kTdup[kvh][qrow:qrow + 64,
                                       b * 512 + kc * 128:
                                       b * 512 + (kc + 1) * 128],
                            qT[qrow:qrow + 64, qp * T + q0:qp * T + q0 + 256],
                            start=True, stop=True)
                    E = p5t.tile([128, 1024], BF16, name="E")
                    nc.scalar.activation(E, sc, AF.Exp, scale=0.125)
                    if prev is not None:
                        emit_ctx(prev)
                        pqh = prev[0][0]
                        if prev[0][1] == 1 and prev[0][2] == 1:
                            pend_renorm = (prev[2], pqh)
                    if pend_renorm is not None:
                        emit_renorm(*pend_renorm)
                        pend_renorm = None
                    prev = ((qh, b, h2), E, cu)
                emit_ctx(prev)
                emit_renorm(prev[2], prev[0][0])

            if upto <= 5:
                with tc.tile_pool(name="anc", bufs=2) as anc:
                    for j in range(KT):
                        a_ = anc.tile([128, T], BF16, name="anc")
                        nc.vector.tensor_copy(a_, ctxT[:, j * T:(j + 1) * T])
                        nc.gpsimd.dma_start(
                            out_d[(j % TC8) * 128:(j % TC8) * 128 + 128, 0:T],
                            a_)
                kv_p.__exit__(None, None, None)
                ctx_p.__exit__(None, None, None)
                return
            kv_p.__exit__(None, None, None)

            # ---- P6: o-proj token-major + residual -> res1 (DRAM f32) ----
            with tc.tile_pool(name="p6t", bufs=2) as p6t, \
                 tc.tile_pool(name="p6w", bufs=4) as p6w, \
                 tc.tile_pool(name="p6ps", bufs=1, space="PSUM") as p6ps:
                for hg in range(4):
                    acc = [p6ps.tile([128, 512], F32, name=f"oacc{tci}")
                           for tci in range(TC8)]
                    for k in range(KT):
                        wblk = p6w.tile([128, 512], BF16, name="woblk")
                        r0 = (hg * KT + k) * 128
                        nc.sync.dma_start(wblk, wo_d[r0:r0 + 128, :])
                        for tci in range(TC8):
                            nc.tensor.matmul(
                                acc[tci],
                                ctxT[:, k * T + tci * 128:
                                     k * T + (tci + 1) * 128],
                                wblk, start=(k == 0), stop=(k == KT - 1))
                    for tci in range(TC8):
                        ot = p6t.tile([128, 512], F32, name="ot")
                        nc.scalar.copy(ot, acc[tci])
                        xs = p6t.tile([128, 512], F32, name="xs")
                        nc.sync.dma_start(
                            xs, x_d[tci * 128:(tci + 1) * 128,
                                    hg * 512:(hg + 1) * 512])
                        r1 = p6t.tile([128, 512], F32, name="r1")
                        nc.vector.tensor_tensor(r1, ot, xs, ADD)
                        nc.sync.dma_start(
                            res1_dram[tci * 128:(tci + 1) * 128,
                                      hg * 512:(hg + 1) * 512], r1)
            ctx_p.__exit__(None, None, None)

            # ---- P7: res1 -> h2T (feat-major bf16), ln2 folded into wg/wu --
            # pool stack: mp (thru P8b) > h2p (thru P8a)
            m_p = tc.tile_pool(name="mp", bufs=1)
            ml = m_p.__enter__()
            M = ml.tile([128, IC * T], BF16, name="M")
            h2_p = tc.tile_pool(name="h2p", bufs=1)
            h2l = h2_p.__enter__()
            h2T = h2l.tile([128, KT * T], BF16, name="h2T")
            with tc.tile_pool(name="p7t", bufs=2) as p7t, \
                 tc.tile_pool(name="p7ps", bufs=4, space="PSUM") as p7ps:
                norm_transpose(lambda i: res1_dram[i * 128:(i + 1) * 128, :],
                               h2T, p7t, p7ps, 4)

            if upto <= 7:
                with tc.tile_pool(name="anc7", bufs=2) as anc:
                    for j in range(KT):
                        a_ = anc.tile([128, T], BF16, name="anc7")
                        nc.vector.tensor_copy(a_, h2T[:, j * T:(j + 1) * T])
                        nc.gpsimd.dma_start(
                            out_d[(j % TC8) * 128:(j % TC8) * 128 + 128, 0:T],
                            a_)
                h2_p.__exit__(None, None, None)
                m_p.__exit__(None, None, None)
                return

            # ---- P8a: gate/up -> M (feat-major bf16, SBUF resident) ----
            with tc.tile_pool(name="p8w", bufs=2) as p8w, \
                 tc.tile_pool(name="p8t", bufs=2) as p8t, \
                 tc.tile_pool(name="p8ps", bufs=2, space="PSUM") as p8ps:
                for ic in range(IC):
                    wgb = p8w.tile([128, 2048], BF16, name="wgb")
                    nc.sync.dma_start(wgb, wg_d[ic * 128:(ic + 1) * 128, :])
                    wub = p8w.tile([128, 2048], BF16, name="wub")
                    nc.sync.dma_start(wub, wu_d[ic * 128:(ic + 1) * 128, :])
                    g_ps = p8ps.tile([128, 1024], F32, name="g_ps")
                    u_ps = p8ps.tile([128, 1024], F32, name="u_ps")
                    # bank-stable bursts: 16 consecutive matmuls into the
                    # same PSUM bank (one accumulation group at a time)
                    for wps, wbl in ((g_ps, wgb), (u_ps, wub)):
                        for th in range(2):
                            for k in range(KT):
                                nc.tensor.matmul(
                                    wps[:, th * 512:(th + 1) * 512],
                                    wbl[:, k * 128:(k + 1) * 128],
                                    h2T[:, k * T + th * 512:
                                        k * T + (th + 1) * 512],
                                    start=(k == 0), stop=(k == KT - 1))
                    for th in range(2):
                        tsl = slice(th * 512, (th + 1) * 512)
                        sg = p8t.tile([128, 512], F32, name="sg")
                        if sim_compat:  # CoreSim lacks Silu
                            sgm = p8t.tile([128, 512], F32, name="sgm")
                            nc.scalar.activation(sgm, g_ps[:, tsl], AF.Sigmoid)
                            gss = p8t.tile([128, 512], F32, name="gss")
                            nc.scalar.copy(gss, g_ps[:, tsl])
                            nc.vector.tensor_tensor(sg, sgm, gss, MULT)
                        else:
                            nc.scalar.activation(sg, g_ps[:, tsl], AF.Silu)
                        su = p8t.tile([128, 512], F32, name="su")
                        nc.scalar.copy(su, u_ps[:, tsl])
                        nc.vector.tensor_tensor(
                            M[:, ic * T + th * 512:ic * T + (th + 1) * 512],
                            sg, su, MULT)
            h2_p.__exit__(None, None, None)

            if upto <= 8:
                with tc.tile_pool(name="anc8", bufs=2) as anc:
                    for j in range(KT):
                        a_ = anc.tile([128, T], BF16, name="anc8")
                        nc.vector.tensor_copy(a_, M[:, j * T:(j + 1) * T])
                        nc.gpsimd.dma_start(
                            out_d[(j % TC8) * 128:(j % TC8) * 128 + 128, 0:T],
                            a_)
                m_p.__exit__(None, None, None)
                return

            # ---- P8b: down-proj token-major, + res1 -> out ----
            with tc.tile_pool(name="p9w", bufs=6) as p9w, \
                 tc.tile_pool(name="p9t", bufs=2) as p9t, \
                 tc.tile_pool(name="p9ps", bufs=1, space="PSUM") as p9ps:
                for hg in range(4):
                    r1s = [p9t.tile([128, 512], F32, name=f"r1b{tci}")
                           for tci in range(TC8)]
                    for tci in range(TC8):
                        nc.sync.dma_start(
                            r1s[tci], res1_dram[tci * 128:(tci + 1) * 128,
                                                hg * 512:(hg + 1) * 512])
                    acc = [p9ps.tile([128, 512], F32, name=f"dacc{tci}")
                           for tci in range(TC8)]
                    for i in range(IC):
                        wdb = p9w.tile([128, 512], BF16, name="wdb")
                        r0 = (hg * IC + i) * 128
                        nc.sync.dma_start(wdb, wd_d[r0:r0 + 128, :])
                        for tci in range(TC8):
                            nc.tensor.matmul(
                                acc[tci],
                                M[:, i * T + tci * 128:i * T + (tci + 1) * 128],
                                wdb, start=(i == 0), stop=(i == IC - 1))
                    for tci in range(TC8):
                        dt_ = p9t.tile([128, 512], F32, name="dt")
                        nc.scalar.copy(dt_, acc[tci])
                        outt = p9t.tile([128, 512], F32, name="outt")
                        nc.vector.tensor_tensor(outt, dt_, r1s[tci], ADD)
                        nc.sync.dma_start(
                            out_d[tci * 128:(tci + 1) * 128,
                                  hg * 512:(hg + 1) * 512], outt)
            m_p.__exit__(None, None, None)

        for _ in range(reps):
            body()

        dram_p.__exit__(None, None, None)
        consts_p.__exit__(None, None, None)

    _split_waits(nc)
    return nc


def _host_tables(pos_ids_core: np.ndarray):
    """cos128/sinS128 [128, T]: feature-major RoPE tables, 2 heads stacked.
    sinS is destination-indexed: rows 0:32 get -sin, rows 32:64 get +sin."""
    pos = pos_ids_core.reshape(-1).astype(np.float64)
    inv_freq = 1.0 / (ROPE_BASE ** (np.arange(0, HD, 2, dtype=np.float64) / HD))
    freqs = pos[None, :] * inv_freq[:, None]   # [32, T]
    cosF = np.cos(freqs)
    sinF = np.sin(freqs)
    cos64 = np.concatenate([cosF, cosF], axis=0)
    sinS64 = np.concatenate([-sinF, sinF], axis=0)
    cos128 = np.concatenate([cos64, cos64], axis=0).astype(np.float32)
    sinS128 = np.concatenate([sinS64, sinS64], axis=0).astype(np.float32)
    return np.ascontiguousarray(cos128), np.ascontiguousarray(sinS128)


_CACHE = {}


def _get_nc(reps: int, upto: int = 9):
    key = (reps, upto)
    if key not in _CACHE:
        _CACHE[key] = build(reps, upto=upto)
    return _CACHE[key]


class _Runner:
    """Persistent PJRT runner: compiles once, keeps inputs resident on device
    so repeated calls don't re-ship replicated weights over axon."""

    def __init__(self, nc, in_maps):
        import jax
        from jax.sharding import Mesh, PartitionSpec, NamedSharding
        from jax.experimental.shard_map import shard_map
        from concourse import bass2jax, mybir as _mb
        bass2jax.install_neuronx_cc_hook()

        n_cores = len(in_maps)
        partition_name = (nc.partition_id_tensor.name
                          if nc.partition_id_tensor else None)
        in_names, out_names, out_avals, zero_outs = [], [], [], []
        for alloc in nc.m.functions[0].allocations:
            if not isinstance(alloc, _mb.MemoryLocationSet):
                continue
            name = alloc.memorylocations[0].name
            if alloc.kind == "ExternalInput":
                if name != partition_name:
                    in_names.append(name)
            elif alloc.kind == "ExternalOutput":
                out_names.append(name)
                shape = tuple(alloc.tensor_shape)
                dtype = _mb.dt.np(alloc.dtype)
                out_avals.append(jax.core.ShapedArray(shape, dtype))
                zero_outs.append(np.zeros(shape, dtype))
        n_params = len(in_names)
        self.out_names = out_names
        self.out_shapes = [tuple(a.shape) for a in out_avals]
        all_in_names = list(in_names) + list(out_names)
        if partition_name is not None:
            all_in_names.append(partition_name)

        def _body(*args):
            operands = list(args)
            if partition_name is not None:
                operands.append(bass2jax.partition_id_tensor())
            outs = bass2jax._bass_exec_p.bind(
                *operands,
                out_avals=tuple(out_avals),
                in_names=tuple(all_in_names),
                out_names=tuple(out_names),
                lowering_input_output_aliases=(),
                sim_require_finite=True,
                sim_require_nnan=True,
                nc=nc,
            )
            return tuple(outs)

        devices = jax.devices()[:n_cores]
        mesh = Mesh(np.asarray(devices), ("core",))
        n_outs = len(out_names)
        in_specs = (PartitionSpec("core"),) * (n_params + n_outs)
        out_specs = (PartitionSpec("core"),) * n_outs
        self.fn = jax.jit(
            shard_map(_body, mesh=mesh, in_specs=in_specs,
                      out_specs=out_specs, check_rep=False),
            keep_unused=True)
        sh = NamedSharding(mesh, PartitionSpec("core"))
        self.dev_in = [
            jax.device_put(
                np.concatenate([np.asarray(in_maps[c][k]) for c in range(n_cores)],
                               axis=0), sh)
            for k in in_names]
        self.dev_zero = [
            jax.device_put(
                np.zeros((n_cores * z.shape[0], *z.shape[1:]), z.dtype), sh)
            for z in zero_outs]
        self.n_cores = n_cores

    def run(self, fetch=True):
        outs = self.fn(*self.dev_in, *self.dev_zero)
        if fetch:
            return [
                {name: np.asarray(outs[i]).reshape(self.n_cores,
                                                   *self.out_shapes[i])[c]
                 for i, name in enumerate(self.out_names)}
                for c in range(self.n_cores)]
        for o in outs:
            o.block_until_ready()
        return None


_RUNNERS = {}


def _prep_weights(wq, wk, wv, wo, wg, wu, wd, ln1_w, ln2_w):
    import ml_dtypes
    bf16 = ml_dtypes.bfloat16
    ln1 = np.asarray(ln1_w, np.float64)
    ln2 = np.asarray(ln2_w, np.float64)
    wqkv = np.concatenate([np.asarray(wq, np.float32),
                           np.asarray(wk, np.float32),
                           np.asarray(wv, np.float32)], axis=1)
    wqkv = (wqkv.astype(np.float64) * ln1[:, None]).astype(np.float32)
    wgf = (np.asarray(wg, np.float64) * ln2[:, None]).astype(np.float32)
    wuf = (np.asarray(wu, np.float64) * ln2[:, None]).astype(np.float32)
    wof = np.asarray(wo, np.float32)
    wdf = np.asarray(wd, np.float32)

    # [mg, k] blocks of [128, 512]
    wqkv_t = np.ascontiguousarray(
        wqkv.reshape(KT, 128, 6, 512).transpose(2, 0, 1, 3)
        .reshape(6 * KT * 128, 512).astype(bf16))
    wo_t = np.ascontiguousarray(
        wof.reshape(KT, 128, 4, 512).transpose(2, 0, 1, 3)
        .reshape(4 * KT * 128, 512).astype(bf16))
    # [ic] slabs of [128, KT*128] (k-major columns)
    wg_t = np.ascontiguousarray(
        wgf.reshape(KT, 128, IC, 128).transpose(2, 1, 0, 3)
        .reshape(IC * 128, KT * 128).astype(bf16))
    wu_t = np.ascontiguousarray(
        wuf.reshape(KT, 128, IC, 128).transpose(2, 1, 0, 3)
        .reshape(IC * 128, KT * 128).astype(bf16))
    # [hg, i] blocks of [128, 512]
    wd_t = np.ascontiguousarray(
        wdf.reshape(IC, 128, 4, 512).transpose(2, 0, 1, 3)
        .reshape(4 * IC * 128, 512).astype(bf16))
    return wqkv_t, wo_t, wg_t, wu_t, wd_t


def kernel(x, pos_ids, wq, wk, wv, wo, wg, wu, wd, ln1_w, ln2_w, reps: int = 1):
    import ml_dtypes
    from concourse.bass_utils import run_bass_kernel_spmd

    x = np.ascontiguousarray(np.asarray(x, dtype=np.float32))
    wqkv_t, wo_t, wg_t, wu_t, wd_t = _prep_weights(
        wq, wk, wv, wo, wg, wu, wd, ln1_w, ln2_w)
    identb = np.eye(128, dtype=ml_dtypes.bfloat16)
    onesm = np.ones((1, 64), np.float32)
    eps = np.full((128, 1), EPS, np.float32)

    pos_ids = np.asarray(pos_ids)
    in_maps = []
    for c in range(N_CORES):
        xs = x[c * BPC:(c + 1) * BPC].reshape(T, HID)
        cos128, sinS128 = _host_tables(pos_ids[c * BPC:(c + 1) * BPC])
        in_maps.append({
            "x": np.ascontiguousarray(xs), "wqkvt": wqkv_t, "wot": wo_t,
            "wgt": wg_t, "wut": wu_t, "wdt": wd_t,
            "cos128": cos128, "sinS128": sinS128, "identb": identb,
            "onesm64": onesm, "eps": eps,
        })

    nc = _get_nc(reps)
    if reps not in _RUNNERS:
        res = run_bass_kernel_spmd(nc, in_maps, core_ids=list(range(N_CORES)))
        results = res.results
        _RUNNERS[reps] = _Runner(nc, in_maps)
    else:
        results = _RUNNERS[reps].run(fetch=True)
    out = np.empty((B, S, HID), np.float32)
    for c in range(N_CORES):
        out[c * BPC:(c + 1) * BPC] = results[c]["out"].reshape(BPC, S, HID)
    return out


def kernel_timed(x, pos_ids, wq, wk, wv, wo, wg, wu, wd, ln1_w, ln2_w,
                 reps: int = 1, n_calls: int = 5):
    """Returns median wall seconds of a device-resident repeated run."""
    import time
    kernel(x, pos_ids, wq, wk, wv, wo, wg, wu, wd, ln1_w, ln2_w, reps=reps)
    r = _RUNNERS[reps]
    r.run(fetch=False)
    times = []
    for _ in range(n_calls):
        t0 = time.time()
        r.run(fetch=False)
        times.append(time.time() - t0)
    return float(np.median(times))


# revision 7
# speedup vs baseline: 1.0459x; 1.0459x over previous
"""Trainium2 Bass kernel v2 for the dense transformer layer (RMSNorm -> GQA
attention -> RMSNorm -> SwiGLU MLP, residuals, RoPE).  b=16,s=512,hid=2048,
nq=32,nkv=8,hd=64,inter=8192, fp32 I/O.

Sharding: data-parallel over batch -- 2 batch elements (1024 tokens) per core
across 8 NeuronCores, no collectives.

v2 changes vs v1:
- Weights shipped as bf16, pre-tiled host-side into DMA-contiguous slabs in
  exact consumption order (no on-chip f32->bf16 DVE conversion, half the DMA).
- ln1/ln2 folded into weight rows host-side.
- o-proj and down-proj produce token-major outputs directly (activation tile
  as lhsT, weight as rhs), accumulating in PSUM: no DRAM accumulator, no
  final transpose phase, residuals added at PSUM drain.
- Attention: one 2048-wide exp per (head, batch); unnormalized ctx + denom
  drained together; softmax renorm deferred to a per-head pass (PE broadcast
  of reciprocal denominators).
- RoPE rotate-halves done as fused swap-multiplies split across DVE and Pool.
- qT/ctxT/M stay SBUF-resident (no DRAM round-trips); res1 (token-major,
  f32) is the only DRAM scratch.
"""

import sys
import numpy as np

sys.path.insert(0, "/opt/trn_rl_repo")

import concourse.bass as bass  # noqa: E402
import concourse.tile as tile  # noqa: E402
from concourse import mybir  # noqa: E402

F32 = mybir.dt.float32
F32R = mybir.dt.float32r
BF16 = mybir.dt.bfloat16
MULT = mybir.AluOpType.mult
ADD = mybir.AluOpType.add
AF = mybir.ActivationFunctionType

N_CORES = 8
B, S, HID = 16, 512, 2048
NQ, NKV, HD, INTER = 32, 8, 64, 8192
T = (B // N_CORES) * S  # tokens per core = 1024
BPC = B // N_CORES      # batch elements per core = 2
KT = HID // 128         # 16 k-tiles of hidden
IC = INTER // 128       # 64 inter col-tiles
TC8 = T // 128          # 8 token chunks
EPS = 1e-6
ROPE_BASE = 10000.0

MAXW = 1  # max sync waits per instruction


def _split_waits(nc):
    k = 0
    for f in nc.m.functions:
        for blk in f.blocks:
            newlist, changed = [], False
            for i in blk.instructions:
                si = i.sync_info
                if si is not None and len(si.on_wait) > MAXW:
                    waits = list(si.on_wait)
                    for w in waits[:-MAXW]:
                        k += 1
                        nop = mybir.InstNoOp(name=f"ws_{k}", ins=[], outs=[])
                        nop.engine = i.engine
                        nop.sync_info = mybir.SyncInfo(on_wait=[w], on_update=[])
                        newlist.append(nop)
                    i.sync_info = mybir.SyncInfo(
                        on_wait=waits[-MAXW:], on_update=list(si.on_update))
                    changed = True
                newlist.append(i)
            if changed:
                blk.instructions = newlist


def build(reps: int = 1, sim_compat: bool = False, upto: int = 9):
    nc = bass.Bass("TRN2", target_bir_lowering=False, debug=False,
                   num_devices=N_CORES)

    x_d = nc.dram_tensor("x", (T, HID), F32, kind="ExternalInput")
    # pre-tiled bf16 weights, [block_row, 512/2048 cols] slabs:
    wqkv_d = nc.dram_tensor("wqkvt", (6 * KT * 128, 512), BF16,
                            kind="ExternalInput")
    wo_d = nc.dram_tensor("wot", (4 * KT * 128, 512), BF16,
                          kind="ExternalInput")
    wg_d = nc.dram_tensor("wgt", (IC * 128, 2048), BF16, kind="ExternalInput")
    wu_d = nc.dram_tensor("wut", (IC * 128, 2048), BF16, kind="ExternalInput")
    wd_d = nc.dram_tensor("wdt", (4 * IC * 128, 512), BF16,
                          kind="ExternalInput")
    cos_d = nc.dram_tensor("cos128", (128, T), F32, kind="ExternalInput")
    sin_d = nc.dram_tensor("sinS128", (128, T), F32, kind="ExternalInput")
    identb_d = nc.dram_tensor("identb", (128, 128), BF16, kind="ExternalInput")
    onesm_d = nc.dram_tensor("onesm64", (1, 64), F32R, kind="ExternalInput")
    eps_d = nc.dram_tensor("eps", (128, 1), F32, kind="ExternalInput")
    out_d = nc.dram_tensor("out", (T, HID), F32, kind="ExternalOutput")

    with tile.TileContext(nc) as tc:
        consts_p = tc.tile_pool(name="consts", bufs=1)
        consts = consts_p.__enter__()
        dram_p = tc.tile_pool(name="drscr", bufs=1, space="DRAM")
        drs = dram_p.__enter__()

        identb = consts.tile([128, 128], BF16)
        nc.sync.dma_start(identb, identb_d[:, :])
        onesm = consts.tile([1, 64], F32R)
        nc.sync.dma_start(onesm, onesm_d[:, :])
        epst = consts.tile([128, 1], F32)
        nc.sync.dma_start(epst, eps_d[:, :])
        cos128 = consts.tile([128, T], F32)
        nc.sync.dma_start(cos128, cos_d[:, :])
        sinS = consts.tile([128, T], F32)
        nc.sync.dma_start(sinS, sin_d[:, :])

        res1_dram = drs.tile([T, HID], F32, name="res1_scr")

        def norm_transpose(src_getter, dst_all, pool, psp, src_f32_bytes):
            """token-major src chunks -> rmsnorm -> feat-major dst tiles.
            src_getter(i) -> (dma_src_ap) for chunk i; dst_all [128, KT*T]."""
            for i in range(TC8):
                x_t = pool.tile([128, HID], F32, name="nx")
                nc.sync.dma_start(x_t, src_getter(i))
                scr = pool.tile([128, HID], BF16, name="nscr")
                ssq = pool.tile([128, 1], F32, name="nssq")
                nc.scalar.activation(scr, x_t, AF.Square, accum_out=ssq)
                istd = pool.tile([128, 1], F32, name="nistd")
                nc.scalar.activation(istd, ssq, AF.Sqrt, bias=epst,
                                     scale=1.0 / HID)
                with nc.allow_low_precision("rms inv-std"):
                    nc.vector.reciprocal(istd, istd)
                h_b = pool.tile([128, HID], BF16, name="nh")
                nc.vector.tensor_scalar(h_b, x_t, istd, None, MULT)
                for jg in range(4):
                    tp = psp.tile([128, 512], BF16, name="ntp")
                    for j4 in range(4):
                        j = jg * 4 + j4
                        nc.tensor.transpose(tp[:, j4 * 128:(j4 + 1) * 128],
                                            h_b[:, j * 128:(j + 1) * 128],
                                            identb)
                    dst = dst_all.rearrange("p (kt t) -> p kt t", kt=KT)
                    nc.scalar.copy(
                        dst[:, jg * 4:jg * 4 + 4, i * 128:(i + 1) * 128],
                        tp.rearrange("p (a b) -> p a b", a=4))

        def body():
            # pool stack (LIFO release): ctxp (thru P6) > kvp (thru P5) >
            # vfp (thru P4) > hTp (thru P3)
            ctx_p = tc.tile_pool(name="ctxp", bufs=1)
            ctxl = ctx_p.__enter__()
            ctxT = ctxl.tile([128, KT * T], BF16, name="ctxT")
            kv_p = tc.tile_pool(name="kvp", bufs=1)
            kvl = kv_p.__enter__()
            qT = kvl.tile([128, KT * T], BF16, name="qT")
            kTdup = [kvl.tile([128, T], BF16, name=f"kTd{j}")
                     for j in range(NKV)]
            v65 = kvl.tile([128, TC8, NKV, 65], BF16, name="v65")
            vf_p = tc.tile_pool(name="vfp", bufs=1)
            vfl = vf_p.__enter__()
            vf = [vfl.tile([128, T], BF16, name=f"vf{j}") for j in range(4)]
            # ---- P1: x -> hT (feat-major bf16), ln1 folded into wqkv ----
            hT_p = tc.tile_pool(name="hTp", bufs=1)
            hTl = hT_p.__enter__()
            hT = hTl.tile([128, KT * T], BF16, name="hT")
            if upto <= 8:
                # serialize reps in truncated builds: chain this rep's hT
                # writes behind the previous rep's out_d anchor writes
                with tc.tile_pool(name="serp", bufs=1) as serp:
                    g_ = serp.tile([128, 128], F32, name="serg")
                    nc.sync.dma_start(g_, out_d[0:128, 0:128])
                    nc.vector.tensor_copy(hT[:, 0:128], g_)
            with tc.tile_pool(name="p1t", bufs=2) as p1t, \
                 tc.tile_pool(name="p1ps", bufs=4, space="PSUM") as p1ps:
                norm_transpose(lambda i: x_d[i * 128:(i + 1) * 128, :],
                               hT, p1t, p1ps, 4)

            # ---- P3: QKV + RoPE; q/k/v feat-major ----
            with tc.tile_pool(name="p3t", bufs=2) as p3t, \
                 tc.tile_pool(name="p3w", bufs=3) as p3w, \
                 tc.tile_pool(name="p3ps", bufs=1, space="PSUM") as p3ps:
                for mg in range(6):
                    ps = [[p3ps.tile([128, 512], F32, name=f"qkv{mi}_{th}")
                           for th in range(2)] for mi in range(4)]
                    for k in range(KT):
                        wblk = p3w.tile([128, 512], BF16, name="wblk")
                        r0 = (mg * KT + k) * 128
                        nc.sync.dma_start(wblk, wqkv_d[r0:r0 + 128, :])
                        for mi in range(4):
                            for th in range(2):
                                nc.tensor.matmul(
                                    ps[mi][th],
                                    wblk[:, mi * 128:(mi + 1) * 128],
                                    hT[:, k * T + th * 512:
                                       k * T + (th + 1) * 512],
                                    start=(k == 0), stop=(k == KT - 1))
                    for mi in range(4):
                        m = mg * 4 + mi
                        for th in range(2):
                            tsl = slice(th * 512, (th + 1) * 512)
                            if m < 20:  # q/k: RoPE
                                qa = p3t.tile([128, 512], F32, name="qa")
                                nc.scalar.copy(qa, ps[mi][th])
                                t1 = p3t.tile([128, 512], F32, name="t1")
                                nc.vector.tensor_tensor(t1, qa, cos128[:, tsl],
                                                        MULT)
                                qsw = p3t.tile([128, 512], F32, name="qsw")
                                for b2 in range(4):
                                    src = slice((b2 ^ 1) * 32,
                                                (b2 ^ 1) * 32 + 32)
                                    dst = slice(b2 * 32, b2 * 32 + 32)
                                    eng = nc.gpsimd if b2 < 2 else nc.scalar
                                    if b2 < 2:
                                        eng.tensor_copy(qsw[dst], qa[src])
                                    else:
                                        nc.scalar.copy(qsw[dst], qa[src])
                                t2 = p3t.tile([128, 512], F32, name="t2")
                                nc.vector.tensor_tensor(t2, qsw,
                                                        sinS[:, tsl], MULT)
                                if m < 16:
                                    nc.vector.tensor_tensor(
                                        qT[:, m * T + th * 512:
                                           m * T + (th + 1) * 512],
                                        t1, t2, ADD)
                                else:
                                    for hh in range(2):
                                        kvh = 2 * (m - 16) + hh
                                        hs = slice(hh * 64, hh * 64 + 64)
                                        for half in range(2):
                                            nc.vector.tensor_tensor(
                                                kTdup[kvh][half * 64:
                                                           half * 64 + 64,
                                                           tsl],
                                                t1[hs], t2[hs], ADD)
                            else:
                                nc.scalar.copy(vf[m - 20][:, tsl], ps[mi][th])
            hT_p.__exit__(None, None, None)

            # ---- P4: v -> token-major v65 (with ones col for denom) ----
            with tc.tile_pool(name="p4ps", bufs=4, space="PSUM") as p4ps:
                # ones column via vf*0+1 (not memset) so it carries a dep on
                # this rep's vf and can't float ahead of the previous rep
                nc.vector.tensor_scalar(v65[:, :, :, 64:65], vf[0][:, 0:64],
                                        0.0, 1.0, MULT, ADD)
                for j in range(4):
                    for tci in range(TC8):
                        tp = p4ps.tile([128, 128], BF16, name="vtp")
                        nc.tensor.transpose(
                            tp, vf[j][:, tci * 128:(tci + 1) * 128], identb)
                        nc.scalar.copy(
                            v65[:, tci, 2 * j:2 * j + 2, 0:64],
                            tp.rearrange("p (a b) -> p a b", a=2))
            vf_p.__exit__(None, None, None)

            # ---- P5: attention; 256-query units, sw-pipelined by 1 ----
            with tc.tile_pool(name="p5t", bufs=3) as p5t, \
                 tc.tile_pool(name="p5u", bufs=3) as p5u, \
                 tc.tile_pool(name="p5psS", bufs=2, space="PSUM") as p5psS, \
                 tc.tile_pool(name="p5psC", bufs=2, space="PSUM") as p5psC, \
                 tc.tile_pool(name="p5psB", bufs=1, space="PSUM") as p5psB:
                def emit_renorm(cu, qh):
                    rden = p5t.tile([1, T], F32R, name="rden")
                    with nc.allow_low_precision("softmax denom"):
                        nc.vector.reciprocal(rden, cu[64:65, :])
                    bc_ps = p5psB.tile([64, T], F32, name="bcp")
                    for th in range(2):
                        nc.tensor.matmul(bc_ps[:, th * 512:(th + 1) * 512],
                                         onesm,
                                         rden[:, th * 512:(th + 1) * 512],
                                         start=True, stop=True)
                    nc.vector.tensor_tensor(
                        ctxT[(qh % 2) * 64:(qh % 2) * 64 + 64,
                             (qh // 2) * T:(qh // 2 + 1) * T],
                        cu[0:64, :], bc_ps, MULT)

                def emit_ctx(u):
                    (qh, b, h2), E, cu = u
                    kvh = qh // 4
                    ctx_ps = p5psC.tile([65, 256], F32, name="ctx")
                    for kc in range(4):
                        nc.tensor.matmul(ctx_ps,
                                         v65[:, b * 4 + kc, kvh, :],
                                         E[:, kc * 256:(kc + 1) * 256],
                                         start=(kc == 0), stop=(kc == 3))
                    q0 = b * 512 + h2 * 256
                    nc.vector.tensor_copy(cu[:, q0:q0 + 256], ctx_ps)

                units = [(qh, b, h2) for qh in range(NQ)
                         for b in range(BPC) for h2 in range(2)]
                prev = None
                pend_renorm = None
                cu = None
                for (qh, b, h2) in units:
                    if b == 0 and h2 == 0:
                        cu = p5u.tile([65, T], BF16, name="cu")
                    kvh = qh // 4
                    qrow = (qh % 2) * 64
                    qp = qh // 2
                    sc = p5psS.tile([128, 1024], F32, name="sc")
                    q0 = b * 512 + h2 * 256
                    for kc in range(4):
                        nc.tensor.matmul(
                            sc[:, kc * 256:(kc + 1) * 256],
                            kTdup[kvh][qrow:qrow + 64,
                                       b * 512 + kc * 128:
                                       b * 512 + (kc + 1) * 128],
                            qT[qrow:qrow + 64, qp * T + q0:qp * T + q0 + 256],
                            start=True, stop=True)
                    E = p5t.tile([128, 1024], BF16, name="E")
                    nc.scalar.activation(E, sc, AF.Exp, scale=0.125)
                    if prev is not None:
                        emit_ctx(prev)
                        pqh = prev[0][0]
                        if prev[0][1] == 1 and prev[0][2] == 1:
                            pend_renorm = (prev[2], pqh)
                    if pend_renorm is not None:
                        emit_renorm(*pend_renorm)
                        pend_renorm = None
                    prev = ((qh, b, h2), E, cu)
                emit_ctx(prev)
                emit_renorm(prev[2], prev[0][0])

            if upto <= 5:
                with tc.tile_pool(name="anc", bufs=2) as anc:
                    for j in range(KT):
                        a_ = anc.tile([128, T], BF16, name="anc")
                        nc.vector.tensor_copy(a_, ctxT[:, j * T:(j + 1) * T])
                        nc.gpsimd.dma_start(
                            out_d[(j % TC8) * 128:(j % TC8) * 128 + 128, 0:T],
                            a_)
                kv_p.__exit__(None, None, None)
                ctx_p.__exit__(None, None, None)
                return
            kv_p.__exit__(None, None, None)

            # ---- P6: o-proj token-major + residual -> res1 (DRAM f32) ----
            with tc.tile_pool(name="p6t", bufs=2) as p6t, \
                 tc.tile_pool(name="p6w", bufs=3) as p6w, \
                 tc.tile_pool(name="p6ps", bufs=1, space="PSUM") as p6ps:
                for hg in range(4):
                    acc = [p6ps.tile([128, 512], F32, name=f"oacc{tci}")
                           for tci in range(TC8)]
                    for k in range(KT):
                        wblk = p6w.tile([128, 512], BF16, name="woblk")
                        r0 = (hg * KT + k) * 128
                        nc.sync.dma_start(wblk, wo_d[r0:r0 + 128, :])
                        for tci in range(TC8):
                            nc.tensor.matmul(
                                acc[tci],
                                ctxT[:, k * T + tci * 128:
                                     k * T + (tci + 1) * 128],
                                wblk, start=(k == 0), stop=(k == KT - 1))
                    for tci in range(TC8):
                        ot = p6t.tile([128, 512], F32, name="ot")
                        nc.scalar.copy(ot, acc[tci])
                        xs = p6t.tile([128, 512], F32, name="xs")
                        nc.sync.dma_start(
                            xs, x_d[tci * 128:(tci + 1) * 128,
                                    hg * 512:(hg + 1) * 512])
                        r1 = p6t.tile([128, 512], F32, name="r1")
                        nc.vector.tensor_tensor(r1, ot, xs, ADD)
                        nc.sync.dma_start(
                            res1_dram[tci * 128:(tci + 1) * 128,
                                      hg * 512:(hg + 1) * 512], r1)
            ctx_p.__exit__(None, None, None)

            # ---- P7: res1 -> h2T (feat-major bf16), ln2 folded into wg/wu --
            # pool stack: mp (thru P8b) > h2p (thru P8a)
            m_p = tc.tile_pool(name="mp", bufs=1)
            ml = m_p.__enter__()
            M = ml.tile([128, IC * T], BF16, name="M")
            h2_p = tc.tile_pool(name="h2p", bufs=1)
            h2l = h2_p.__enter__()
            h2T = h2l.tile([128, KT * T], BF16, name="h2T")
            with tc.tile_pool(name="p7t", bufs=2) as p7t, \
                 tc.tile_pool(name="p7ps", bufs=4, space="PSUM") as p7ps:
                norm_transpose(lambda i: res1_dram[i * 128:(i + 1) * 128, :],
                               h2T, p7t, p7ps, 4)

            if upto <= 7:
                with tc.tile_pool(name="anc7", bufs=2) as anc:
                    for j in range(KT):
                        a_ = anc.tile([128, T], BF16, name="anc7")
                        nc.vector.tensor_copy(a_, h2T[:, j * T:(j + 1) * T])
                        nc.gpsimd.dma_start(
                            out_d[(j % TC8) * 128:(j % TC8) * 128 + 128, 0:T],
                            a_)
                h2_p.__exit__(None, None, None)
                m_p.__exit__(None, None, None)
                return

            # ---- P8a: gate/up -> M (feat-major bf16, SBUF resident) ----
            with tc.tile_pool(name="p8w", bufs=2) as p8w, \
                 tc.tile_pool(name="p8t", bufs=2) as p8t, \
                 tc.tile_pool(name="p8ps", bufs=2, space="PSUM") as p8ps:
                for ic in range(IC):
                    wgb = p8w.tile([128, 2048], BF16, name="wgb")
                    nc.sync.dma_start(wgb, wg_d[ic * 128:(ic + 1) * 128, :])
                    wub = p8w.tile([128, 2048], BF16, name="wub")
                    nc.sync.dma_start(wub, wu_d[ic * 128:(ic + 1) * 128, :])
                    g_ps = p8ps.tile([128, 1024], F32, name="g_ps")
                    u_ps = p8ps.tile([128, 1024], F32, name="u_ps")
                    for k in range(KT):
                        for wps, wbl in ((g_ps, wgb), (u_ps, wub)):
                            for th in range(2):
                                nc.tensor.matmul(
                                    wps[:, th * 512:(th + 1) * 512],
                                    wbl[:, k * 128:(k + 1) * 128],
                                    h2T[:, k * T + th * 512:
                                        k * T + (th + 1) * 512],
                                    start=(k == 0), stop=(k == KT - 1))
                    for th in range(2):
                        tsl = slice(th * 512, (th + 1) * 512)
                        sg = p8t.tile([128, 512], F32, name="sg")
                        if sim_compat:  # CoreSim lacks Silu
                            sgm = p8t.tile([128, 512], F32, name="sgm")
                            nc.scalar.activation(sgm, g_ps[:, tsl], AF.Sigmoid)
                            gss = p8t.tile([128, 512], F32, name="gss")
                            nc.scalar.copy(gss, g_ps[:, tsl])
                            nc.vector.tensor_tensor(sg, sgm, gss, MULT)
                        else:
                            nc.scalar.activation(sg, g_ps[:, tsl], AF.Silu)
                        su = p8t.tile([128, 512], F32, name="su")
                        nc.scalar.copy(su, u_ps[:, tsl])
                        nc.vector.tensor_tensor(
                            M[:, ic * T + th * 512:ic * T + (th + 1) * 512],
                            sg, su, MULT)
            h2_p.__exit__(None, None, None)

            if upto <= 8:
                with tc.tile_pool(name="anc8", bufs=2) as anc:
                    for j in range(KT):
                        a_ = anc.tile([128, T], BF16, name="anc8")
                        nc.vector.tensor_copy(a_, M[:, j * T:(j + 1) * T])
                        nc.gpsimd.dma_start(
                            out_d[(j % TC8) * 128:(j % TC8) * 128 + 128, 0:T],
                            a_)
                m_p.__exit__(None, None, None)
                return

            # ---- P8b: down-proj token-major, + res1 -> out ----
            with tc.tile_pool(name="p9w", bufs=6) as p9w, \
                 tc.tile_pool(name="p9t", bufs=2) as p9t, \
                 tc.tile_pool(name="p9ps", bufs=1, space="PSUM") as p9ps:
                for hg in range(4):
                    acc = [p9ps.tile([128, 512], F32, name=f"dacc{tci}")
                           for tci in range(TC8)]
                    for i in range(IC):
                        wdb = p9w.tile([128, 512], BF16, name="wdb")
                        r0 = (hg * IC + i) * 128
                        nc.sync.dma_start(wdb, wd_d[r0:r0 + 128, :])
                        for tci in range(TC8):
                            nc.tensor.matmul(
                                acc[tci],
                                M[:, i * T + tci * 128:i * T + (tci + 1) * 128],
                                wdb, start=(i == 0), stop=(i == IC - 1))
                    for tci in range(TC8):
                        dt_ = p9t.tile([128, 512], F32, name="dt")
                        nc.scalar.copy(dt_, acc[tci])
                        r1 = p9t.tile([128, 512], F32, name="r1b")
                        nc.sync.dma_start(
                            r1, res1_dram[tci * 128:(tci + 1) * 128,
                                          hg * 512:(hg + 1) * 512])
                        outt = p9t.tile([128, 512], F32, name="outt")
                        nc.vector.tensor_tensor(outt, dt_, r1, ADD)
                        nc.sync.dma_start(
                            out_d[tci * 128:(tci + 1) * 128,
                                  hg * 512:(hg + 1) * 512], outt)
            m_p.__exit__(None, None, None)

        for _ in range(reps):
            body()

        dram_p.__exit__(None, None, None)
        consts_p.__exit__(None, None, None)

    _split_waits(nc)
    return nc


def _host_tables(pos_ids_core: np.ndarray):
    """cos128/sinS128 [128, T]: feature-major RoPE tables, 2 heads stacked.
    sinS is destination-indexed: rows 0:32 get -sin, rows 32:64 get +sin."""
    pos = pos_ids_core.reshape(-1).astype(np.float64)
    inv_freq = 1.0 / (ROPE_BASE ** (np.arange(0, HD, 2, dtype=np.float64) / HD))
    freqs = pos[None, :] * inv_freq[:, None]   # [32, T]
    cosF = np.cos(freqs)
    sinF = np.sin(freqs)
    cos64 = np.concatenate([cosF, cosF], axis=0)
    sinS64 = np.concatenate([-sinF, sinF], axis=0)
    cos128 = np.concatenate([cos64, cos64], axis=0).astype(np.float32)
    sinS128 = np.concatenate([sinS64, sinS64], axis=0).astype(np.float32)
    return np.ascontiguousarray(cos128), np.ascontiguousarray(sinS128)


_CACHE = {}


def _get_nc(reps: int, upto: int = 9):
    key = (reps, upto)
    if key not in _CACHE:
        _CACHE[key] = build(reps, upto=upto)
    return _CACHE[key]


class _Runner:
    """Persistent PJRT runner: compiles once, keeps inputs resident on device
    so repeated calls don't re-ship replicated weights over axon."""

    def __init__(self, nc, in_maps):
        import jax
        from jax.sharding import Mesh, PartitionSpec, NamedSharding
        from jax.experimental.shard_map import shard_map
        from concourse import bass2jax, mybir as _mb
        bass2jax.install_neuronx_cc_hook()

        n_cores = len(in_maps)
        partition_name = (nc.partition_id_tensor.name
                          if nc.partition_id_tensor else None)
        in_names, out_names, out_avals, zero_outs = [], [], [], []
        for alloc in nc.m.functions[0].allocations:
            if not isinstance(alloc, _mb.MemoryLocationSet):
                continue
            name = alloc.memorylocations[0].name
            if alloc.kind == "ExternalInput":
                if name != partition_name:
                    in_names.append(name)
            elif alloc.kind == "ExternalOutput":
                out_names.append(name)
                shape = tuple(alloc.tensor_shape)
                dtype = _mb.dt.np(alloc.dtype)
                out_avals.append(jax.core.ShapedArray(shape, dtype))
                zero_outs.append(np.zeros(shape, dtype))
        n_params = len(in_names)
        self.out_names = out_names
        self.out_shapes = [tuple(a.shape) for a in out_avals]
        all_in_names = list(in_names) + list(out_names)
        if partition_name is not None:
            all_in_names.append(partition_name)

        def _body(*args):
            operands = list(args)
            if partition_name is not None:
                operands.append(bass2jax.partition_id_tensor())
            outs = bass2jax._bass_exec_p.bind(
                *operands,
                out_avals=tuple(out_avals),
                in_names=tuple(all_in_names),
                out_names=tuple(out_names),
                lowering_input_output_aliases=(),
                sim_require_finite=True,
                sim_require_nnan=True,
                nc=nc,
            )
            return tuple(outs)

        devices = jax.devices()[:n_cores]
        mesh = Mesh(np.asarray(devices), ("core",))
        n_outs = len(out_names)
        in_specs = (PartitionSpec("core"),) * (n_params + n_outs)
        out_specs = (PartitionSpec("core"),) * n_outs
        self.fn = jax.jit(
            shard_map(_body, mesh=mesh, in_specs=in_specs,
                      out_specs=out_specs, check_rep=False),
            keep_unused=True)
        sh = NamedSharding(mesh, PartitionSpec("core"))
        self.dev_in = [
            jax.device_put(
                np.concatenate([np.asarray(in_maps[c][k]) for c in range(n_cores)],
                               axis=0), sh)
            for k in in_names]
        self.dev_zero = [
            jax.device_put(
                np.zeros((n_cores * z.shape[0], *z.shape[1:]), z.dtype), sh)
            for z in zero_outs]
        self.n_cores = n_cores

    def run(self, fetch=True):
        outs = self.fn(*self.dev_in, *self.dev_zero)
        if fetch:
            return [
                {name: np.asarray(outs[i]).reshape(self.n_cores,
                                                   *self.out_shapes[i])[c]
                 for i, name in enumerate(self.out_names)}
                for c in range(self.n_cores)]
        for o in outs:
            o.block_until_ready()
        return None


_RUNNERS = {}


def _prep_weights(wq, wk, wv, wo, wg, wu, wd, ln1_w, ln2_w):
    import ml_dtypes
    bf16 = ml_dtypes.bfloat16
    ln1 = np.asarray(ln1_w, np.float64)
    ln2 = np.asarray(ln2_w, np.float64)
    wqkv = np.concatenate([np.asarray(wq, np.float32),
                           np.asarray(wk, np.float32),
                           np.asarray(wv, np.float32)], axis=1)
    wqkv = (wqkv.astype(np.float64) * ln1[:, None]).astype(np.float32)
    wgf = (np.asarray(wg, np.float64) * ln2[:, None]).astype(np.float32)
    wuf = (np.asarray(wu, np.float64) * ln2[:, None]).astype(np.float32)
    wof = np.asarray(wo, np.float32)
    wdf = np.asarray(wd, np.float32)

    # [mg, k] blocks of [128, 512]
    wqkv_t = np.ascontiguousarray(
        wqkv.reshape(KT, 128, 6, 512).transpose(2, 0, 1, 3)
        .reshape(6 * KT * 128, 512).astype(bf16))
    wo_t = np.ascontiguousarray(
        wof.reshape(KT, 128, 4, 512).transpose(2, 0, 1, 3)
        .reshape(4 * KT * 128, 512).astype(bf16))
    # [ic] slabs of [128, KT*128] (k-major columns)
    wg_t = np.ascontiguousarray(
        wgf.reshape(KT, 128, IC, 128).transpose(2, 1, 0, 3)
        .reshape(IC * 128, KT * 128).astype(bf16))
    wu_t = np.ascontiguousarray(
        wuf.reshape(KT, 128, IC, 128).transpose(2, 1, 0, 3)
        .reshape(IC * 128, KT * 128).astype(bf16))
    # [hg, i] blocks of [128, 512]
    wd_t = np.ascontiguousarray(
        wdf.reshape(IC, 128, 4, 512).transpose(2, 0, 1, 3)
        .reshape(4 * IC * 128, 512).astype(bf16))
    return wqkv_t, wo_t, wg_t, wu_t, wd_t


def kernel(x, pos_ids, wq, wk, wv, wo, wg, wu, wd, ln1_w, ln2_w, reps: int = 1):
    import ml_dtypes
    from concourse.bass_utils import run_bass_kernel_spmd

    x = np.ascontiguousarray(np.asarray(x, dtype=np.float32))
    wqkv_t, wo_t, wg_t, wu_t, wd_t = _prep_weights(
        wq, wk, wv, wo, wg, wu, wd, ln1_w, ln2_w)
    identb = np.eye(128, dtype=ml_dtypes.bfloat16)
    onesm = np.ones((1, 64), np.float32)
    eps = np.full((128, 1), EPS, np.float32)

    pos_ids = np.asarray(pos_ids)
    in_maps = []
    for c in range(N_CORES):
        xs = x[c * BPC:(c + 1) * BPC].reshape(T, HID)
        cos128, sinS128 = _host_tables(pos_ids[c * BPC:(c + 1) * BPC])
        in_maps.append({
            "x": np.ascontiguousarray(xs), "wqkvt": wqkv_t, "wot": wo_t,
            "wgt": wg_t, "wut": wu_t, "wdt": wd_t,
            "cos128": cos128, "sinS128": sinS128, "identb": identb,
            "onesm64": onesm, "eps": eps,
        })

    nc = _get_nc(reps)
    if reps not in _RUNNERS:
        res = run_bass_kernel_spmd(nc, in_maps, core_ids=list(range(N_CORES)))
        results = res.results
        _RUNNERS[reps] = _Runner(nc, in_maps)
    else:
        results = _RUNNERS[reps].run(fetch=True)
    out = np.empty((B, S, HID), np.float32)
    for c in range(N_CORES):
        out[c * BPC:(c + 1) * BPC] = results[c]["out"].reshape(BPC, S, HID)
    return out


def kernel_timed(x, pos_ids, wq, wk, wv, wo, wg, wu, wd, ln1_w, ln2_w,
                 reps: int = 1, n_calls: int = 5):
    """Returns median wall seconds of a device-resident repeated run."""
    import time
    kernel(x, pos_ids, wq, wk, wv, wo, wg, wu, wd, ln1_w, ln2_w, reps=reps)
    r = _RUNNERS[reps]
    r.run(fetch=False)
    times = []
    for _ in range(n_calls):
        t0 = time.time()
        r.run(fetch=False)
        times.append(time.time() - t0)
    return float(np.median(times))
